# revision 25
# baseline (speedup 1.0000x reference)
"""GCN2 (GCNII) message-passing kernel for 8 Trainium2 NeuronCores.

Strategy (1D node sharding, spec sharding_hint):
- Nodes padded to NPAD = 8*NBLK*128 and sharded contiguously across 8 cores.
- Edges (incl. self-loops) partitioned by destination core, sorted by
  destination block (128 dests), each block's edge run padded to a multiple
  of 128 with the per-block tile count made uniform across cores (SPMD).
- Per layer, per core: the full "support" table [NPAD, 64] fp16 lives in
  DRAM (partition-major row remap); an indirect DMA gathers one 128-byte
  row per edge; the segment-sum is a one-hot matmul: P[e,d] =
  (iota==col_rel)*norm built in one DVE tensor_scalar, PE accumulates
  P.T @ G into a per-block PSUM tile.
- z = agg + initial with initial = h0 @ (w2 + alpha*I) (identity folded on
  host), support = h @ (w1 + I). BatchNorm stats via ones-matmul + tiny
  AllReduce; updated support shards are AllGathered into the table.
- fp16 is used for the table/gather, one-hot P and PE inputs (validated
  absmax-rel err ~7e-4 vs f32 reference); f32 everywhere else.
"""
import math
from contextlib import ExitStack

import numpy as np

import concourse.bass as bass
import concourse.bacc as bacc
import concourse.tile as tile
from concourse import mybir
from concourse.bass_utils import run_bass_kernel_spmd
from concourse.masks import make_identity

DT = mybir.dt
F16 = DT.float16
F32 = DT.float32

NC = 8
BLK = 128
ALPHA = 0.5
EPS = 1e-5
CHUNK_BLKS = 5  # agg PSUM (5+1 boundary banks) + 2 stats banks = 8
DEBUG = False
TIME_REPEATS = 0
EXEC_NS = None
EXEC_TIMES = None
TRACE = False
TRACE_DIR = None
LAST = None


# ---------------------------------------------------------------- host prep
def _host_prep(edge_index, n_nodes, npad, shard, nblk, chunk_blks):
    """Per-core edge arrays + SPMD-uniform gather/matmul schedule.

    Table rows use the partition-major remap rr = c*shard + p*nblk + b.
    dma_gather (int16 idx, 256B elems) reads a [npad//4, 4*64] wide view:
    idx w = rr//4 selects a 4-node row; the call's column window cc2=(rr%4)//2
    picks a 2-node (256B) element; parity rr%2 picks the 64-col half, chosen
    per tile (edges are sorted by (cc2, par, block) within each chunk).

    Returns dict with:
      gidx  [NC, 128, gcols]  int16  (16-wrapped, replicated to 128 parts)
      edat  [NC, 128, 2*ntiles] f32  (col_rel | norm, per tile slot)
      chunks: list of dicts:
         calls: [(gi0_cols, ncols16, num_idxs, col128_0), ...]  (per cc2)
         tiles: [(gcol_elem_off, [(blk, var, start, stop), ...]), ...]
      ntiles, gcols
    """
    e = np.asarray(edge_index)
    row = np.concatenate([e[0], np.arange(n_nodes, dtype=np.int64)])
    col = np.concatenate([e[1], np.arange(n_nodes, dtype=np.int64)])
    deg = np.bincount(col, minlength=n_nodes).astype(np.float64)
    dinv = np.where(deg > 0, deg**-0.5, 0.0)
    norm = (dinv[row] * dinv[col]).astype(np.float32)

    core = (col // shard).astype(np.int64)
    blk = ((col % shard) // BLK).astype(np.int64)
    crel_g = (col % shard).astype(np.int64)          # 0..shard-1 (block-rel later)

    c_src = row // shard
    r_src = row % shard
    rr = c_src * shard + (r_src % BLK) * nblk + (r_src // BLK)
    w_all = (rr // 4).astype(np.int64)
    cc2_all = ((rr % 4) // 2).astype(np.int64)
    par_all = (rr % 2).astype(np.int64)

    chunks_b = []
    b0 = 0
    while b0 < nblk:
        chunks_b.append((b0, min(b0 + chunk_blks, nblk)))
        b0 = min(b0 + chunk_blks, nblk)

    chunk_of = np.zeros(nblk, dtype=np.int64)
    for ci, (cb0, cb1) in enumerate(chunks_b):
        chunk_of[cb0:cb1] = ci

    key_chunk = chunk_of[blk]
    # global sort: (core, chunk, cc2, par, blk)
    order = np.lexsort((blk, par_all, cc2_all, key_chunk, core))
    S = dict(row=row[order], w=w_all[order], cc2=cc2_all[order],
             par=par_all[order], blk=blk[order], core=core[order],
             chunk=key_chunk[order], crel=crel_g[order], nrm=norm[order])

    # run lengths per (core, chunk, cc2, par)
    nchunk = len(chunks_b)
    cnt = np.zeros((NC, nchunk, 2, 2), dtype=np.int64)
    np.add.at(cnt, (S["core"], S["chunk"], S["cc2"], S["par"]), 1)
    # uniform across cores, rounded to 128
    run_len = (-(-cnt.max(axis=0) // BLK) * BLK)     # [nchunk, 2, 2]
    run_len = np.maximum(run_len, BLK)

    # build padded flat arrays per core
    ntiles = int(run_len.sum()) // BLK
    tot = ntiles * BLK
    p_w = np.zeros((NC, tot), dtype=np.int16)
    p_crel = np.full((NC, tot), 20000.0, dtype=np.float32)
    p_nrm = np.zeros((NC, tot), dtype=np.float32)
    p_blk = np.full((NC, tot), -1, dtype=np.int64)

    run_off = np.zeros((nchunk, 2, 2), dtype=np.int64)
    acc = 0
    for ci in range(nchunk):
        for cc2 in range(2):
            for par in range(2):
                run_off[ci, cc2, par] = acc
                acc += run_len[ci, cc2, par]

    # slot per edge: run offset + rank within (core, chunk, cc2, par)
    grp_key = (((S["core"] * nchunk + S["chunk"]) * 2 + S["cc2"]) * 2 + S["par"])
    grp_change = np.concatenate([[True], grp_key[1:] != grp_key[:-1]])
    grp_start = np.where(grp_change)[0]
    rank = np.arange(len(grp_key)) - np.repeat(
        grp_start, np.diff(np.concatenate([grp_start, [len(grp_key)]])))
    slot = run_off[S["chunk"], S["cc2"], S["par"]] + rank
    p_w[S["core"], slot] = S["w"].astype(np.int16)
    p_nrm[S["core"], slot] = S["nrm"]
    p_blk[S["core"], slot] = S["blk"]

    # schedule: per chunk -> calls and tiles
    chunks = []
    gcol128 = 0          # cumulative 128-col G elements within chunk resets
    gidx_cols = 0
    for ci, (cb0, cb1) in enumerate(chunks_b):
        calls = []
        tiles = []
        ccol = 0         # G column-128 index within this chunk
        for cc2 in range(2):
            nidx = int(run_len[ci, cc2, 0] + run_len[ci, cc2, 1])
            calls.append(dict(gi0=gidx_cols, ncols16=nidx // 16,
                              num_idxs=nidx, col128_0=ccol))
            gidx_cols += nidx // 16
            for par in range(2):
                off = int(run_off[ci, cc2, par])
                nt = int(run_len[ci, cc2, par]) // BLK
                for t in range(nt):
                    tiles.append(dict(slot0=off + t * BLK,
                                      gcol=ccol * BLK + par * 64))
                    ccol += 1
        chunks.append(dict(calls=calls, tiles=tiles, b0=cb0, b1=cb1,
                           ncol128=ccol))

    # per-tile block spans + col_rel bases + start/stop flags
    # (identical across cores by construction of run_len; block membership
    #  differs per core, so compute the union across cores per tile)
    first_tile = {}
    last_tile = {}
    tile_global = 0
    for ch in chunks:
        for td in ch["tiles"]:
            s0 = td["slot0"]
            blks = p_blk[:, s0:s0 + BLK]
            real = blks >= 0
            if real.any():
                b_lo = int(blks[real].min())
                b_hi = int(blks[real].max())
            else:
                b_lo = b_hi = ch["b0"]
            assert b_hi - b_lo < 4, "tile spans too many blocks"
            td["b_lo"] = b_lo
            td["pairs"] = list(range(b_lo, b_hi + 1))
            td["tid"] = tile_global
            for b in td["pairs"]:
                if b not in first_tile:
                    first_tile[b] = tile_global
                last_tile[b] = tile_global
            tile_global += 1
    assert tile_global == ntiles

    for ch in chunks:
        for td in ch["tiles"]:
            td["flags"] = [(b, b - td["b_lo"],
                            first_tile[b] == td["tid"],
                            last_tile[b] == td["tid"]) for b in td["pairs"]]

    # per-core col_rel relative to tile b_lo
    edat = np.zeros((NC, BLK, 2 * ntiles), dtype=np.float32)
    gidx = np.zeros((NC, BLK, gidx_cols), dtype=np.int16)
    crel_full = np.full((NC, tot), 20000.0, dtype=np.float32)
    crel_full[S["core"], slot] = S["crel"].astype(np.float32)
    ti = 0
    for ch in chunks:
        for td in ch["tiles"]:
            s0 = td["slot0"]
            cr = crel_full[:, s0:s0 + BLK] - (td["b_lo"] * BLK)
            cr[p_blk[:, s0:s0 + BLK] < 0] = 20000.0
            edat[:, :, ti] = cr
            edat[:, :, ntiles + ti] = p_nrm[:, s0:s0 + BLK]
            ti += 1
    assert ti == ntiles

    # gidx: per call, 16-wrapped layout replicated across 128 partitions
    for ch in chunks:
        for ca in ch["calls"]:
            # idx array for this call = slots [base, base+num_idxs)
            base = None
        # compute from run offsets: call cc2 covers runs (cc2,0),(cc2,1)
    for ci, ch in enumerate(chunks):
        for cc2 in range(2):
            ca = ch["calls"][cc2]
            base = int(run_off[ci, cc2, 0])
            n = ca["num_idxs"]
            vals = p_w[:, base:base + n]                    # [NC, n]
            wrap = vals.reshape(NC, n // 16, 16).transpose(0, 2, 1)  # [NC,16,n/16]
            gi0 = ca["gi0"]
            for rep in range(8):
                gidx[:, rep * 16:(rep + 1) * 16, gi0:gi0 + n // 16] = wrap
    return dict(gidx=gidx, edat=edat, chunks=chunks, ntiles=ntiles,
                gcols=gidx_cols)


# ---------------------------------------------------------------- program
def _build(nc, cfg):
    fin = cfg["fin"]
    hid = cfg["hid"]
    outd = cfg["outd"]
    nlay = cfg["nlay"]
    shard = cfg["shard"]
    nblk = cfg["nblk"]
    npad = cfg["npad"]
    n_nodes = cfg["n"]
    ntiles = cfg["ntiles"]
    outp = max(outd, 1)

    xT = nc.declare_dram_parameter("xT", [fin, shard], F16, isOutput=False)
    gidx = nc.declare_dram_parameter("gidx", [BLK, cfg["gcols"]], DT.int16, isOutput=False)
    edat = nc.declare_dram_parameter("edat", [BLK, 2 * ntiles], F32, isOutput=False)
    wi = nc.declare_dram_parameter("wi", [fin, hid], F16, isOutput=False)
    w1p = nc.declare_dram_parameter("w1p", [nlay, hid, hid], F16, isOutput=False)
    w2p = nc.declare_dram_parameter("w2p", [nlay, hid, hid], F16, isOutput=False)
    gb = nc.declare_dram_parameter("gb", [nlay, 2, hid], F32, isOutput=False)
    wo = nc.declare_dram_parameter("wo", [hid, outp], F16, isOutput=False)
    bvec = nc.declare_dram_parameter("bvec", [2, hid], F32, isOutput=False)  # bi|bo
    msk = nc.declare_dram_parameter("msk", [BLK, nblk], F32, isOutput=False)
    out_p = nc.declare_dram_parameter("out", [BLK, nblk * outp], F32, isOutput=True)
    dbg = cfg.get("debug", False)
    if dbg:
        dbg_h = nc.declare_dram_parameter("dbg_h", [BLK, nblk * hid], F32, isOutput=True)
        dbg_sup = nc.declare_dram_parameter("dbg_sup", [BLK, nblk * hid], F16, isOutput=True)
        dbg_g = nc.declare_dram_parameter("dbg_g", [BLK, cfg["max_chunk_cols"] * BLK], F16, isOutput=True)
        dbg_z = nc.declare_dram_parameter("dbg_z", [BLK, nblk * hid], F32, isOutput=True)
        dbg_st = nc.declare_dram_parameter("dbg_st", [1, 2 * hid], F32, isOutput=True)

    core_ids = list(range(NC))
    inv_n = 1.0 / float(n_nodes)

    with tile.TileContext(nc) as tc, ExitStack() as ctx:
        const = ctx.enter_context(tc.tile_pool(name="const", bufs=1))
        gpool = ctx.enter_context(tc.tile_pool(name="gpool", bufs=2))
        ppool = ctx.enter_context(tc.tile_pool(name="ppool", bufs=6))
        tpool = ctx.enter_context(tc.tile_pool(name="tpool", bufs=4))
        htpool = ctx.enter_context(tc.tile_pool(name="htpool", bufs=4))
        dram = ctx.enter_context(tc.tile_pool(name="dram", bufs=1, space="DRAM"))

        tables = [dram.tile([npad, hid], F16, addr_space="Shared",
                            name=f"table{i}") for i in range(nlay)]
        sup_local = dram.tile([shard, hid], F16)
        stats_in = dram.tile([1, 2 * hid], F32)
        stats_outs = [dram.tile([1, 2 * hid], F32, addr_space="Shared",
                               name=f"statso{i}") for i in range(nlay)]

        # ---- constants
        iotas = []
        for v in range(4):
            iota_i = const.tile([BLK, BLK], DT.int16, tag=f"ioti{v}")
            nc.gpsimd.iota(iota_i, pattern=[[1, BLK]], base=v * BLK,
                           channel_multiplier=0)
            iota_v = const.tile([BLK, BLK], F16, tag=f"iotf{v}")
            nc.vector.tensor_copy(iota_v, iota_i)
            iotas.append(iota_v)
        ident = const.tile([BLK, BLK], F32)
        make_identity(nc, ident)
        ones = const.tile([BLK, 1], F32)
        nc.vector.memset(ones, 1.0)

        wi_sb = const.tile([fin, hid], F16)
        nc.sync.dma_start(out=wi_sb, in_=wi[:, :])
        w1_sb = const.tile([hid, nlay * hid], F16)
        w2_sb = const.tile([hid, nlay * hid], F16)
        for l in range(nlay):
            nc.sync.dma_start(out=w1_sb[:, l * hid:(l + 1) * hid], in_=w1p[l, :, :])
            nc.sync.dma_start(out=w2_sb[:, l * hid:(l + 1) * hid], in_=w2p[l, :, :])
        wo_sb = const.tile([hid, outp], F16)
        nc.sync.dma_start(out=wo_sb, in_=wo[:, :])
        gb_sb = const.tile([1, nlay * 2 * hid], F32)
        nc.sync.dma_start(out=gb_sb, in_=gb.rearrange("l t h -> (l t h)")[None, :])
        brow = const.tile([1, 2 * hid], F32)
        nc.sync.dma_start(out=brow, in_=bvec.rearrange("a h -> (a h)")[None, :])
        bi_bc = const.tile([BLK, hid], F32)
        nc.gpsimd.partition_broadcast(bi_bc, brow[:1, 0:hid])
        bo_bc = const.tile([BLK, hid], F32)
        nc.gpsimd.partition_broadcast(bo_bc, brow[:1, hid:2 * hid])
        mask_sb = const.tile([BLK, nblk], F32)
        nc.sync.dma_start(out=mask_sb, in_=msk[:, :])

        gidx_sb = const.tile([BLK, cfg["gcols"]], DT.int16)
        nc.sync.dma_start(out=gidx_sb, in_=gidx[:, :])
        edat_sb = const.tile([BLK, 2 * ntiles], F32)
        nc.sync.dma_start(out=edat_sb, in_=edat[:, :])

        xT_sb = const.tile([fin, shard], F16)
        nc.sync.dma_start(out=xT_sb, in_=xT[:, :])

        # ---- persistent state
        hprev = const.tile([BLK, nblk * hid], F32)
        zinit = const.tile([BLK, nblk * hid], F32)   # initial_l, then z
        h0T = const.tile([hid, nblk * BLK], F16)
        supsend = const.tile([BLK, nblk * hid], F16)
        outsb = const.tile([BLK, nblk * outp], F32)
        stats_sb = const.tile([1, 2 * hid], F32)

        def h0T_slice(b):
            return h0T[:, b * BLK:(b + 1) * BLK]

        def transpose_to_f16(ps_pool, src_sl):
            """[128, hid] f32 SBUF -> [hid, 128] f16 SBUF via PE transpose."""
            pt = ps_pool.tile([hid, BLK], F32, tag="u")
            nc.tensor.transpose(out=pt, in_=src_sl, identity=ident)
            ht = htpool.tile([hid, BLK], F16, tag="ht")
            nc.vector.tensor_copy(ht, pt)
            return ht

        # ---- input layer: h = relu(x @ Wi + bi) * mask; h0T; support0; initial0
        with tc.tile_pool(name="in_ps", bufs=8, space="PSUM") as in_ps:
            for b in range(nblk):
                ph = in_ps.tile([BLK, hid], F32, tag="u")
                nc.tensor.matmul(ph, lhsT=xT_sb[:, b * BLK:(b + 1) * BLK],
                                 rhs=wi_sb, start=True, stop=True)
                hsl = hprev[:, b * hid:(b + 1) * hid]
                nc.vector.tensor_tensor(out=hsl, in0=ph, in1=bi_bc,
                                        op=mybir.AluOpType.add)
                nc.vector.tensor_scalar(out=hsl, in0=hsl, scalar1=0.0,
                                        scalar2=mask_sb[:, b:b + 1],
                                        op0=mybir.AluOpType.max,
                                        op1=mybir.AluOpType.mult)
                ht = transpose_to_f16(in_ps, hsl)
                nc.vector.tensor_copy(h0T_slice(b), ht)
                ps = in_ps.tile([BLK, hid], F32, tag="u")
                nc.tensor.matmul(ps, lhsT=ht, rhs=w1_sb[:, 0:hid],
                                 start=True, stop=True)
                nc.vector.tensor_copy(supsend[:, b * hid:(b + 1) * hid], ps)
                pi = in_ps.tile([BLK, hid], F32, tag="u")
                nc.tensor.matmul(pi, lhsT=ht, rhs=w2_sb[:, 0:hid],
                                 start=True, stop=True)
                nc.vector.tensor_copy(zinit[:, b * hid:(b + 1) * hid], pi)

        if dbg:
            nc.sync.dma_start(out=dbg_h[:, :], in_=hprev)
            nc.sync.dma_start(out=dbg_sup[:, :], in_=supsend)
        nc.sync.dma_start(out=sup_local.opt(), in_=supsend)
        nc.gpsimd.collective_compute(
            "AllGather", mybir.AluOpType.bypass, replica_groups=[core_ids],
            ins=[sup_local.opt()], outs=[tables[0].opt()])

        # ---- layers
        table_wide = None  # built lazily from table AP

        for l in range(nlay):
            lay_ctx = ExitStack()
            agg_ps = lay_ctx.enter_context(
                tc.tile_pool(name=f"agg{l}", bufs=CHUNK_BLKS + 1, space="PSUM"))
            st_ps = lay_ctx.enter_context(
                tc.tile_pool(name=f"st{l}", bufs=1, space="PSUM"))
            st_sum = st_ps.tile([1, hid], F32, tag="ssum")
            st_sq = st_ps.tile([1, hid], F32, tag="ssq")
            evict_cnt = 0
            for ch in cfg["chunks"]:
                g = gpool.tile([BLK, cfg["max_chunk_cols"] * BLK], F16, tag="g")
                tw = tables[l][:, :].rearrange("(w q) f -> w (q f)", q=4)
                for cc2, ca in enumerate(ch["calls"]):
                    a0 = ca["col128_0"] * BLK
                    nc.gpsimd.dma_gather(
                        out_ap=g[:, a0:a0 + ca["num_idxs"]].rearrange(
                            "p (t f) -> p t f", f=BLK),
                        in_ap=tw[:, cc2 * BLK:(cc2 + 1) * BLK],
                        idxs_ap=gidx_sb[:, ca["gi0"]:ca["gi0"] + ca["ncols16"]],
                        num_idxs=ca["num_idxs"],
                        num_idxs_reg=ca["num_idxs"],
                        elem_size=BLK,
                        elem_step=4 * hid,
                        single_packet=(ca["num_idxs"] <= 1024),
                    )
                agg_tiles = {}
                if dbg and l == 0 and ch is cfg["chunks"][0]:
                    nc.sync.dma_start(
                        out=dbg_g[:, :ch["ncol128"] * BLK],
                        in_=g[:, :ch["ncol128"] * BLK])
                for td in ch["tiles"]:
                    gc = td["gcol"]
                    ti = td["tid"]
                    for (b, var, st, sp) in td["flags"]:
                        pt = ppool.tile([BLK, BLK], F16, tag="p")
                        nc.vector.tensor_scalar(
                            out=pt, in0=iotas[var],
                            scalar1=edat_sb[:, ti:ti + 1],
                            scalar2=edat_sb[:, ntiles + ti:ntiles + ti + 1],
                            op0=mybir.AluOpType.is_equal,
                            op1=mybir.AluOpType.mult)
                        if st:
                            agg_tiles[b] = agg_ps.tile([BLK, hid], F32,
                                                       tag="agg",
                                                       name=f"agg_b{b}")
                        nc.tensor.matmul(agg_tiles[b], lhsT=pt,
                                         rhs=g[:, gc:gc + hid],
                                         start=st, stop=sp)
                        if sp:
                            zsl = zinit[:, b * hid:(b + 1) * hid]
                            nc.vector.tensor_tensor(out=zsl, in0=agg_tiles[b],
                                                    in1=zsl,
                                                    op=mybir.AluOpType.add)
                            zsq = tpool.tile([BLK, hid], F32, tag="zsq")
                            nc.vector.tensor_tensor(out=zsq, in0=zsl, in1=zsl,
                                                    op=mybir.AluOpType.mult)
                            nc.tensor.matmul(st_sum, lhsT=ones[:, :1], rhs=zsl,
                                             start=(evict_cnt == 0),
                                             stop=(evict_cnt == nblk - 1))
                            nc.tensor.matmul(st_sq, lhsT=ones[:, :1], rhs=zsq,
                                             start=(evict_cnt == 0),
                                             stop=(evict_cnt == nblk - 1))
                            evict_cnt += 1
                            del agg_tiles[b]

            # stats allreduce
            nc.vector.tensor_copy(stats_sb[:1, 0:hid], st_sum)
            nc.vector.tensor_copy(stats_sb[:1, hid:2 * hid], st_sq)
            lay_ctx.close()
            nc.sync.dma_start(out=stats_in.opt(), in_=stats_sb)
            nc.gpsimd.collective_compute(
                "AllReduce", mybir.AluOpType.add, replica_groups=[core_ids],
                ins=[stats_in.opt()], outs=[stats_outs[l].opt()])
            nc.sync.dma_start(out=stats_sb, in_=stats_outs[l].opt())

            mt = tpool.tile([1, hid], F32, tag="mt")
            nc.vector.tensor_scalar(out=mt, in0=stats_sb[:1, 0:hid],
                                    scalar1=inv_n, scalar2=None,
                                    op0=mybir.AluOpType.mult)
            vt = tpool.tile([1, hid], F32, tag="vt")
            nc.vector.tensor_scalar(out=vt, in0=stats_sb[:1, hid:2 * hid],
                                    scalar1=inv_n, scalar2=None,
                                    op0=mybir.AluOpType.mult)
            mm = tpool.tile([1, hid], F32, tag="mm")
            nc.vector.tensor_tensor(out=mm, in0=mt, in1=mt,
                                    op=mybir.AluOpType.mult)
            nc.vector.tensor_tensor(out=vt, in0=vt, in1=mm,
                                    op=mybir.AluOpType.subtract)
            rstd = tpool.tile([1, hid], F32, tag="rstd")
            nc.vector.tensor_scalar(out=vt, in0=vt, scalar1=EPS, scalar2=None,
                                    op0=mybir.AluOpType.add)
            nc.vector.reciprocal(rstd, vt)
            nc.scalar.activation(out=rstd, in_=rstd,
                                 func=mybir.ActivationFunctionType.Sqrt)
            scl = tpool.tile([1, hid], F32, tag="scl")
            nc.vector.tensor_tensor(out=scl, in0=gb_sb[:1, (l * 2) * hid:(l * 2 + 1) * hid],
                                    in1=rstd, op=mybir.AluOpType.mult)
            sht = tpool.tile([1, hid], F32, tag="sht")
            nc.vector.tensor_tensor(out=sht, in0=mt, in1=scl,
                                    op=mybir.AluOpType.mult)
            nc.vector.tensor_tensor(out=sht, in0=gb_sb[:1, (l * 2 + 1) * hid:(l * 2 + 2) * hid],
                                    in1=sht, op=mybir.AluOpType.subtract)
            if dbg and l == 0:
                nc.sync.dma_start(out=dbg_z[:, :], in_=zinit)
                nc.sync.dma_start(out=dbg_st[:, :], in_=stats_sb)
            scl_bc = tpool.tile([BLK, hid], F32, tag="sclbc")
            nc.gpsimd.partition_broadcast(scl_bc, scl[:1, :])
            sht_bc = tpool.tile([BLK, hid], F32, tag="shtbc")
            nc.gpsimd.partition_broadcast(sht_bc, sht[:1, :])

            # update h, produce next support/initial (or final output)
            up_ctx = ExitStack()
            up_ps = up_ctx.enter_context(
                tc.tile_pool(name=f"up{l}", bufs=8, space="PSUM"))
            for b in range(nblk):
                zsl = zinit[:, b * hid:(b + 1) * hid]
                hsl = hprev[:, b * hid:(b + 1) * hid]
                t1_ = tpool.tile([BLK, hid], F32, tag="t1")
                nc.vector.tensor_tensor(out=t1_, in0=zsl, in1=scl_bc,
                                        op=mybir.AluOpType.mult)
                nc.vector.tensor_tensor(out=t1_, in0=t1_, in1=sht_bc,
                                        op=mybir.AluOpType.add)
                nc.vector.tensor_scalar(out=t1_, in0=t1_, scalar1=0.0,
                                        scalar2=None, op0=mybir.AluOpType.max)
                nc.vector.tensor_tensor(out=hsl, in0=t1_, in1=hsl,
                                        op=mybir.AluOpType.add)
                ht = transpose_to_f16(up_ps, hsl)
                if l < nlay - 1:
                    ps = up_ps.tile([BLK, hid], F32, tag="u")
                    nc.tensor.matmul(ps, lhsT=ht,
                                     rhs=w1_sb[:, (l + 1) * hid:(l + 2) * hid],
                                     start=True, stop=True)
                    nc.vector.tensor_copy(supsend[:, b * hid:(b + 1) * hid], ps)
                    pi = up_ps.tile([BLK, hid], F32, tag="u")
                    nc.tensor.matmul(pi, lhsT=h0T_slice(b),
                                     rhs=w2_sb[:, (l + 1) * hid:(l + 2) * hid],
                                     start=True, stop=True)
                    nc.vector.tensor_copy(zinit[:, b * hid:(b + 1) * hid], pi)
                else:
                    po = up_ps.tile([BLK, outp], F32, tag="u")
                    nc.tensor.matmul(po, lhsT=ht[:, :], rhs=wo_sb,
                                     start=True, stop=True)
                    nc.vector.tensor_tensor(
                        out=outsb[:, b * outp:(b + 1) * outp], in0=po,
                        in1=bo_bc[:, :outp], op=mybir.AluOpType.add)
            up_ctx.close()
            if l < nlay - 1:
                nc.sync.dma_start(out=sup_local.opt(), in_=supsend)
                nc.gpsimd.collective_compute(
                    "AllGather", mybir.AluOpType.bypass,
                    replica_groups=[core_ids],
                    ins=[sup_local.opt()], outs=[tables[l + 1].opt()])

        nc.sync.dma_start(out=out_p[:, :], in_=outsb)
    return nc


# ------------------------------------------------------------- timed runner
def _run_spmd_timed(nc, in_maps, n_cores, repeats):
    """Mirror of bass2jax.run_bass_via_pjrt with repeat timing (no donation,
    inputs pre-staged on device)."""
    import jax
    import time
    from jax.sharding import Mesh, PartitionSpec, NamedSharding
    from jax.experimental.shard_map import shard_map
    from concourse import bass2jax

    bass2jax.install_neuronx_cc_hook()
    partition_name = (nc.partition_id_tensor.name
                     if nc.partition_id_tensor else None)
    in_names, out_names, out_avals, zero_outs = [], [], [], []
    for alloc in nc.m.functions[0].allocations:
        if not isinstance(alloc, mybir.MemoryLocationSet):
            continue
        name = alloc.memorylocations[0].name
        if alloc.kind == "ExternalInput":
            if name != partition_name:
                in_names.append(name)
        elif alloc.kind == "ExternalOutput":
            shape = tuple(alloc.tensor_shape)
            dtype = mybir.dt.np(alloc.dtype)
            out_names.append(name)
            out_avals.append(jax.core.ShapedArray(shape, dtype))
            zero_outs.append(np.zeros(shape, dtype))
    n_params = len(in_names)
    in_names_full = list(in_names) + list(out_names)
    if partition_name is not None:
        in_names_full.append(partition_name)

    def _body(*args):
        operands = list(args)
        if partition_name is not None:
            operands.append(bass2jax.partition_id_tensor())
        outs = bass2jax._bass_exec_p.bind(
            *operands, out_avals=tuple(out_avals),
            in_names=tuple(in_names_full), out_names=tuple(out_names),
            lowering_input_output_aliases=(),
            sim_require_finite=True, sim_require_nnan=True, nc=nc)
        return tuple(outs)

    devices = jax.devices()[:n_cores]
    mesh = Mesh(np.asarray(devices), ("core",))
    spec = PartitionSpec("core")
    n_outs = len(out_avals)
    sharded = jax.jit(shard_map(
        _body, mesh=mesh, in_specs=(spec,) * (n_params + n_outs),
        out_specs=(spec,) * n_outs, check_rep=False), keep_unused=True)
    concat_in = [
        np.concatenate([np.asarray(in_maps[c][nm]) for c in range(n_cores)],
                       axis=0)
        for nm in in_names]
    concat_zeros = [np.zeros((n_cores * z.shape[0], *z.shape[1:]), z.dtype)
                    for z in zero_outs]
    sh = NamedSharding(mesh, spec)
    dev_in = [jax.device_put(a, sh) for a in concat_in + concat_zeros]
    for a in dev_in:
        a.block_until_ready()
    out_arrs = sharded(*dev_in)
    jax.block_until_ready(out_arrs)
    times = []
    for _ in range(repeats):
        t0 = time.perf_counter()
        o = sharded(*dev_in)
        jax.block_until_ready(o)
        times.append(time.perf_counter() - t0)
    exec_ns = int(min(times) * 1e9) if times else None
    results = [
        {nm: np.asarray(out_arrs[i]).reshape(
            n_cores, *out_avals[i].shape)[c]
         for i, nm in enumerate(out_names)}
        for c in range(n_cores)]
    return results, exec_ns, times


# ---------------------------------------------------------------- entry
def kernel(**inputs):
    x = np.asarray(inputs["x"], np.float32)
    edge_index = np.asarray(inputs["edge_index"])
    Wi = np.asarray(inputs["Wi"], np.float32)
    bi = np.asarray(inputs["bi"], np.float32)
    w1 = np.asarray(inputs["w1"], np.float32)
    w2 = np.asarray(inputs["w2"], np.float32)
    gamma = np.asarray(inputs["gamma"], np.float32)
    beta = np.asarray(inputs["beta"], np.float32)
    Wo = np.asarray(inputs["Wo"], np.float32)
    bo = np.asarray(inputs["bo"], np.float32)

    n_nodes, fin = x.shape
    hid = Wi.shape[1]
    nlay = w1.shape[0]
    outd = Wo.shape[1]
    nblk = -(-n_nodes // (NC * BLK))
    shard = nblk * BLK
    npad = NC * shard

    assert npad % 4 == 0
    prep = _host_prep(edge_index, n_nodes, npad, shard, nblk, CHUNK_BLKS)
    ntiles = prep["ntiles"]
    max_chunk_cols = max(ch["ncol128"] for ch in prep["chunks"])

    cfg = dict(fin=fin, hid=hid, outd=outd, nlay=nlay, shard=shard, nblk=nblk,
               npad=npad, n=n_nodes, ntiles=ntiles, chunks=prep["chunks"],
               gcols=prep["gcols"], max_chunk_cols=max_chunk_cols, debug=DEBUG)

    # host-side tensor prep
    xpad = np.zeros((npad, fin), np.float32)
    xpad[:n_nodes] = x
    w1p = (w1 + np.eye(hid, dtype=np.float32)).astype(np.float16)
    w2p = (w2 + ALPHA * np.eye(hid, dtype=np.float32)).astype(np.float16)
    gb = np.stack([gamma, beta], axis=1).astype(np.float32)      # [L,2,H]
    bvec = np.zeros((2, hid), np.float32)
    bvec[0] = bi
    bvec[1, :outd] = bo
    mask = np.zeros((npad,), np.float32)
    mask[:n_nodes] = 1.0

    in_maps = []
    for c in range(NC):
        xs = xpad[c * shard:(c + 1) * shard].astype(np.float16)
        mk = mask[c * shard:(c + 1) * shard].reshape(nblk, BLK).T.copy()
        in_maps.append({
            "xT": np.ascontiguousarray(xs.T),
            "gidx": prep["gidx"][c],
            "edat": prep["edat"][c],
            "wi": Wi.astype(np.float16),
            "w1p": w1p, "w2p": w2p, "gb": gb,
            "wo": Wo.astype(np.float16), "bvec": bvec,
            "msk": np.ascontiguousarray(mk),
        })

    nc = bacc.Bacc("TRN2", target_bir_lowering=False, debug=False,
                   num_devices=NC)
    _build(nc, cfg)
    nc.compile()
    global LAST, EXEC_NS, EXEC_TIMES
    if TIME_REPEATS > 0:
        results, EXEC_NS, EXEC_TIMES = _run_spmd_timed(
            nc, in_maps, NC, TIME_REPEATS)

        class _R:
            pass
        res = _R()
        res.results = results
        res.exec_time_ns = EXEC_NS
        res.mean_exec_time_ns = None
        LAST = res
    else:
        res = run_bass_kernel_spmd(nc, in_maps, list(range(NC)),
                                   trace=TRACE, tmpdir=TRACE_DIR)
        LAST = res

    outp = max(outd, 1)
    parts = []
    for c in range(NC):
        arr = res.results[c]["out"]                    # [128, nblk*outp]
        arr = arr.reshape(BLK, nblk, outp).transpose(1, 0, 2).reshape(shard, outp)
        parts.append(arr[:, :outd])
    full = np.concatenate(parts, axis=0)[:n_nodes]
    return full.astype(np.float32)


# revision 30
# speedup vs baseline: 17.9100x; 17.9100x over previous
"""GCN2 (GCNII) message-passing kernel for 8 Trainium2 NeuronCores.

Strategy (1D node sharding, spec sharding_hint):
- Nodes padded to NPAD = 8*NBLK*128 and sharded contiguously across 8 cores.
- Edges (incl. self-loops) partitioned by destination core, sorted by
  destination block (128 dests), each block's edge run padded to a multiple
  of 128 with the per-block tile count made uniform across cores (SPMD).
- Per layer, per core: the full "support" table [NPAD, 64] fp16 lives in
  DRAM (partition-major row remap); an indirect DMA gathers one 128-byte
  row per edge; the segment-sum is a one-hot matmul: P[e,d] =
  (iota==col_rel)*norm built in one DVE tensor_scalar, PE accumulates
  P.T @ G into a per-block PSUM tile.
- z = agg + initial with initial = h0 @ (w2 + alpha*I) (identity folded on
  host), support = h @ (w1 + I). BatchNorm stats via ones-matmul + tiny
  AllReduce; updated support shards are AllGathered into the table.
- fp16 is used for the table/gather, one-hot P and PE inputs (validated
  absmax-rel err ~7e-4 vs f32 reference); f32 everywhere else.
"""
import math
from contextlib import ExitStack

import numpy as np

import concourse.bass as bass
import concourse.bacc as bacc
import concourse.tile as tile
from concourse import mybir
from concourse.bass_utils import run_bass_kernel_spmd
from concourse.masks import make_identity

DT = mybir.dt
F16 = DT.float16
F32 = DT.float32

NC = 8
BLK = 128
ALPHA = 0.5
EPS = 1e-5
CHUNK_BLKS = 5  # agg PSUM (5+1 boundary banks) + 2 stats banks = 8
DEBUG = False
BUILD_ONLY = False
NO_COLLECTIVES = False
TIME_REPEATS = 0
EXEC_NS = None
EXEC_TIMES = None
TRACE = False
TRACE_DIR = None
LAST = None


# ---------------------------------------------------------------- host prep
def _host_prep(edge_index, n_nodes, npad, shard, nblk, chunk_blks):
    """Per-core edge arrays + SPMD-uniform gather/matmul schedule.

    Table rows use the partition-major remap rr = c*shard + p*nblk + b.
    dma_gather (int16 idx, 256B elems) reads a [npad//4, 4*64] wide view:
    idx w = rr//4 selects a 4-node row; the call's column window cc2=(rr%4)//2
    picks a 2-node (256B) element; parity rr%2 picks the 64-col half, chosen
    per tile (edges are sorted by (cc2, par, block) within each chunk).

    Returns dict with:
      gidx  [NC, 128, gcols]  int16  (16-wrapped, replicated to 128 parts)
      edat  [NC, 128, 2*ntiles] f32  (col_rel | norm, per tile slot)
      chunks: list of dicts:
         calls: [(gi0_cols, ncols16, num_idxs, col128_0), ...]  (per cc2)
         tiles: [(gcol_elem_off, [(blk, var, start, stop), ...]), ...]
      ntiles, gcols
    """
    e = np.asarray(edge_index)
    row = np.concatenate([e[0], np.arange(n_nodes, dtype=np.int64)])
    col = np.concatenate([e[1], np.arange(n_nodes, dtype=np.int64)])
    deg = np.bincount(col, minlength=n_nodes).astype(np.float64)
    dinv = np.where(deg > 0, deg**-0.5, 0.0)
    norm = (dinv[row] * dinv[col]).astype(np.float32)

    core = (col // shard).astype(np.int64)
    blk = ((col % shard) // BLK).astype(np.int64)
    crel_g = (col % shard).astype(np.int64)          # 0..shard-1 (block-rel later)

    c_src = row // shard
    r_src = row % shard
    rr = c_src * shard + (r_src % BLK) * nblk + (r_src // BLK)
    w_all = (rr // 4).astype(np.int64)
    cc2_all = ((rr % 4) // 2).astype(np.int64)
    par_all = (rr % 2).astype(np.int64)

    chunks_b = []
    b0 = 0
    while b0 < nblk:
        chunks_b.append((b0, min(b0 + chunk_blks, nblk)))
        b0 = min(b0 + chunk_blks, nblk)

    chunk_of = np.zeros(nblk, dtype=np.int64)
    for ci, (cb0, cb1) in enumerate(chunks_b):
        chunk_of[cb0:cb1] = ci

    key_chunk = chunk_of[blk]
    # global sort: (core, chunk, cc2, par, blk)
    order = np.lexsort((blk, par_all, cc2_all, key_chunk, core))
    S = dict(row=row[order], w=w_all[order], cc2=cc2_all[order],
             par=par_all[order], blk=blk[order], core=core[order],
             chunk=key_chunk[order], crel=crel_g[order], nrm=norm[order])

    # run lengths per (core, chunk, cc2, par)
    nchunk = len(chunks_b)
    cnt = np.zeros((NC, nchunk, 2, 2), dtype=np.int64)
    np.add.at(cnt, (S["core"], S["chunk"], S["cc2"], S["par"]), 1)
    # uniform across cores, rounded to 128
    run_len = (-(-cnt.max(axis=0) // BLK) * BLK)     # [nchunk, 2, 2]
    run_len = np.maximum(run_len, BLK)

    # build padded flat arrays per core
    ntiles = int(run_len.sum()) // BLK
    tot = ntiles * BLK
    p_w = np.zeros((NC, tot), dtype=np.int16)
    p_crel = np.full((NC, tot), 20000.0, dtype=np.float32)
    p_nrm = np.zeros((NC, tot), dtype=np.float32)
    p_blk = np.full((NC, tot), -1, dtype=np.int64)

    run_off = np.zeros((nchunk, 2, 2), dtype=np.int64)
    acc = 0
    for ci in range(nchunk):
        for cc2 in range(2):
            for par in range(2):
                run_off[ci, cc2, par] = acc
                acc += run_len[ci, cc2, par]

    # slot per edge: run offset + rank within (core, chunk, cc2, par)
    grp_key = (((S["core"] * nchunk + S["chunk"]) * 2 + S["cc2"]) * 2 + S["par"])
    grp_change = np.concatenate([[True], grp_key[1:] != grp_key[:-1]])
    grp_start = np.where(grp_change)[0]
    rank = np.arange(len(grp_key)) - np.repeat(
        grp_start, np.diff(np.concatenate([grp_start, [len(grp_key)]])))
    slot = run_off[S["chunk"], S["cc2"], S["par"]] + rank
    p_w[S["core"], slot] = S["w"].astype(np.int16)
    p_nrm[S["core"], slot] = S["nrm"]
    p_blk[S["core"], slot] = S["blk"]

    # schedule: per chunk -> calls and tiles
    chunks = []
    gcol128 = 0          # cumulative 128-col G elements within chunk resets
    gidx_cols = 0
    for ci, (cb0, cb1) in enumerate(chunks_b):
        calls = []
        tiles = []
        ccol = 0         # G column-128 index within this chunk
        for cc2 in range(2):
            nidx = int(run_len[ci, cc2, 0] + run_len[ci, cc2, 1])
            calls.append(dict(gi0=gidx_cols, ncols16=nidx // 16,
                              num_idxs=nidx, col128_0=ccol))
            gidx_cols += nidx // 16
            for par in range(2):
                off = int(run_off[ci, cc2, par])
                nt = int(run_len[ci, cc2, par]) // BLK
                for t in range(nt):
                    tiles.append(dict(slot0=off + t * BLK,
                                      gcol=ccol * BLK + par * 64))
                    ccol += 1
        chunks.append(dict(calls=calls, tiles=tiles, b0=cb0, b1=cb1,
                           ncol128=ccol))

    # per-tile block spans + col_rel bases + start/stop flags
    # (identical across cores by construction of run_len; block membership
    #  differs per core, so compute the union across cores per tile)
    first_tile = {}
    last_tile = {}
    tile_global = 0
    for ch in chunks:
        for td in ch["tiles"]:
            s0 = td["slot0"]
            blks = p_blk[:, s0:s0 + BLK]
            real = blks >= 0
            if real.any():
                b_lo = int(blks[real].min())
                b_hi = int(blks[real].max())
            else:
                b_lo = b_hi = ch["b0"]
            assert b_hi - b_lo < 4, "tile spans too many blocks"
            td["b_lo"] = b_lo
            td["pairs"] = list(range(b_lo, b_hi + 1))
            td["tid"] = tile_global
            for b in td["pairs"]:
                if b not in first_tile:
                    first_tile[b] = tile_global
                last_tile[b] = tile_global
            tile_global += 1
    assert tile_global == ntiles

    for ch in chunks:
        for td in ch["tiles"]:
            td["flags"] = [(b, b - td["b_lo"],
                            first_tile[b] == td["tid"],
                            last_tile[b] == td["tid"]) for b in td["pairs"]]

    # per-core col_rel relative to tile b_lo
    edat = np.zeros((NC, BLK, 2 * ntiles), dtype=np.float32)
    gidx = np.zeros((NC, BLK, gidx_cols), dtype=np.int16)
    crel_full = np.full((NC, tot), 20000.0, dtype=np.float32)
    crel_full[S["core"], slot] = S["crel"].astype(np.float32)
    ti = 0
    for ch in chunks:
        for td in ch["tiles"]:
            s0 = td["slot0"]
            cr = crel_full[:, s0:s0 + BLK] - (td["b_lo"] * BLK)
            cr[p_blk[:, s0:s0 + BLK] < 0] = 20000.0
            edat[:, :, ti] = cr
            edat[:, :, ntiles + ti] = p_nrm[:, s0:s0 + BLK]
            ti += 1
    assert ti == ntiles

    # gidx: per call, 16-wrapped layout replicated across 128 partitions
    for ch in chunks:
        for ca in ch["calls"]:
            # idx array for this call = slots [base, base+num_idxs)
            base = None
        # compute from run offsets: call cc2 covers runs (cc2,0),(cc2,1)
    for ci, ch in enumerate(chunks):
        for cc2 in range(2):
            ca = ch["calls"][cc2]
            base = int(run_off[ci, cc2, 0])
            n = ca["num_idxs"]
            vals = p_w[:, base:base + n]                    # [NC, n]
            wrap = vals.reshape(NC, n // 16, 16).transpose(0, 2, 1)  # [NC,16,n/16]
            gi0 = ca["gi0"]
            for rep in range(8):
                gidx[:, rep * 16:(rep + 1) * 16, gi0:gi0 + n // 16] = wrap
    return dict(gidx=gidx, edat=edat, chunks=chunks, ntiles=ntiles,
                gcols=gidx_cols)


# ---------------------------------------------------------------- program
def _build(nc, cfg):
    fin = cfg["fin"]
    hid = cfg["hid"]
    outd = cfg["outd"]
    nlay = cfg["nlay"]
    shard = cfg["shard"]
    nblk = cfg["nblk"]
    npad = cfg["npad"]
    n_nodes = cfg["n"]
    ntiles = cfg["ntiles"]
    outp = max(outd, 1)

    xT = nc.declare_dram_parameter("xT", [fin, shard], F16, isOutput=False)
    gidx = nc.declare_dram_parameter("gidx", [BLK, cfg["gcols"]], DT.int16, isOutput=False)
    edat = nc.declare_dram_parameter("edat", [BLK, 2 * ntiles], F32, isOutput=False)
    wi = nc.declare_dram_parameter("wi", [fin, hid], F16, isOutput=False)
    w1p = nc.declare_dram_parameter("w1p", [nlay, hid, hid], F16, isOutput=False)
    w2p = nc.declare_dram_parameter("w2p", [nlay, hid, hid], F16, isOutput=False)
    gb = nc.declare_dram_parameter("gb", [nlay, 2, hid], F32, isOutput=False)
    wo = nc.declare_dram_parameter("wo", [hid, outp], F16, isOutput=False)
    bvec = nc.declare_dram_parameter("bvec", [2, hid], F32, isOutput=False)  # bi|bo
    msk = nc.declare_dram_parameter("msk", [BLK, nblk], F32, isOutput=False)
    out_p = nc.declare_dram_parameter("out", [BLK, nblk * outp], F32, isOutput=True)
    dbg = cfg.get("debug", False)
    if dbg:
        dbg_h = nc.declare_dram_parameter("dbg_h", [BLK, nblk * hid], F32, isOutput=True)
        dbg_sup = nc.declare_dram_parameter("dbg_sup", [BLK, nblk * hid], F16, isOutput=True)
        dbg_g = nc.declare_dram_parameter("dbg_g", [BLK, cfg["max_chunk_cols"] * BLK], F16, isOutput=True)
        dbg_z = nc.declare_dram_parameter("dbg_z", [BLK, nblk * hid], F32, isOutput=True)
        dbg_st = nc.declare_dram_parameter("dbg_st", [1, 2 * hid], F32, isOutput=True)

    core_ids = list(range(NC))
    inv_n = 1.0 / float(n_nodes)

    with tile.TileContext(nc) as tc, ExitStack() as ctx:
        const = ctx.enter_context(tc.tile_pool(name="const", bufs=1))
        gpool = ctx.enter_context(tc.tile_pool(name="gpool", bufs=2))
        ppool = ctx.enter_context(tc.tile_pool(name="ppool", bufs=8))
        tpool = ctx.enter_context(tc.tile_pool(name="tpool", bufs=4))
        htpool = ctx.enter_context(tc.tile_pool(name="htpool", bufs=4))
        dram = ctx.enter_context(tc.tile_pool(name="dram", bufs=1, space="DRAM"))

        tables = [dram.tile([npad, hid], F16, addr_space="Shared",
                            name=f"table{i}") for i in range(nlay)]
        sup_local = dram.tile([shard, hid], F16)
        stats_in = dram.tile([1, 2 * hid], F32)
        stats_outs = [dram.tile([1, 2 * hid], F32, addr_space="Shared",
                               name=f"statso{i}") for i in range(nlay)]

        # ---- constants
        iotas = []
        for v in range(4):
            iota_i = const.tile([BLK, BLK], DT.int16, tag=f"ioti{v}")
            nc.gpsimd.iota(iota_i, pattern=[[1, BLK]], base=v * BLK,
                           channel_multiplier=0)
            iota_v = const.tile([BLK, BLK], F16, tag=f"iotf{v}")
            nc.vector.tensor_copy(iota_v, iota_i)
            iotas.append(iota_v)
        ident = const.tile([BLK, BLK], F32)
        make_identity(nc, ident)
        ones = const.tile([BLK, 1], F32)
        nc.vector.memset(ones, 1.0)

        wi_sb = const.tile([fin, hid], F16)
        nc.sync.dma_start(out=wi_sb, in_=wi[:, :])
        w1_sb = const.tile([hid, nlay * hid], F16)
        w2_sb = const.tile([hid, nlay * hid], F16)
        for l in range(nlay):
            nc.sync.dma_start(out=w1_sb[:, l * hid:(l + 1) * hid], in_=w1p[l, :, :])
            nc.sync.dma_start(out=w2_sb[:, l * hid:(l + 1) * hid], in_=w2p[l, :, :])
        wo_sb = const.tile([hid, outp], F16)
        nc.sync.dma_start(out=wo_sb, in_=wo[:, :])
        gb_sb = const.tile([1, nlay * 2 * hid], F32)
        nc.sync.dma_start(out=gb_sb, in_=gb.rearrange("l t h -> (l t h)")[None, :])
        brow = const.tile([1, 2 * hid], F32)
        nc.sync.dma_start(out=brow, in_=bvec.rearrange("a h -> (a h)")[None, :])
        bi_bc = const.tile([BLK, hid], F32)
        nc.gpsimd.partition_broadcast(bi_bc, brow[:1, 0:hid])
        bo_bc = const.tile([BLK, hid], F32)
        nc.gpsimd.partition_broadcast(bo_bc, brow[:1, hid:2 * hid])
        mask_sb = const.tile([BLK, nblk], F32)
        nc.sync.dma_start(out=mask_sb, in_=msk[:, :])

        gidx_sb = const.tile([BLK, cfg["gcols"]], DT.int16)
        nc.sync.dma_start(out=gidx_sb, in_=gidx[:, :])
        edat_sb = const.tile([BLK, 2 * ntiles], F32)
        nc.sync.dma_start(out=edat_sb, in_=edat[:, :])

        xT_sb = const.tile([fin, shard], F16)
        nc.sync.dma_start(out=xT_sb, in_=xT[:, :])

        # ---- persistent state
        hprev = const.tile([BLK, nblk * hid], F32)
        zinit = const.tile([BLK, nblk * hid], F32)   # initial_l, then z
        h0T = const.tile([hid, nblk * BLK], F16)
        supsend = const.tile([BLK, nblk * hid], F16)
        outsb = const.tile([BLK, nblk * outp], F32)
        stats_sb = const.tile([1, 2 * hid], F32)

        def h0T_slice(b):
            return h0T[:, b * BLK:(b + 1) * BLK]

        def transpose_to_f16(ps_pool, src_sl):
            """[128, hid] f32 SBUF -> [hid, 128] f16 SBUF via PE transpose."""
            pt = ps_pool.tile([hid, BLK], F32, tag="u")
            nc.tensor.transpose(out=pt, in_=src_sl, identity=ident)
            ht = htpool.tile([hid, BLK], F16, tag="ht")
            nc.vector.tensor_copy(ht, pt)
            return ht

        # ---- input layer: h = relu(x @ Wi + bi) * mask; h0T; support0; initial0
        with tc.tile_pool(name="in_ps", bufs=8, space="PSUM") as in_ps:
            for b in range(nblk):
                ph = in_ps.tile([BLK, hid], F32, tag="u")
                nc.tensor.matmul(ph, lhsT=xT_sb[:, b * BLK:(b + 1) * BLK],
                                 rhs=wi_sb, start=True, stop=True)
                hsl = hprev[:, b * hid:(b + 1) * hid]
                nc.vector.tensor_tensor(out=hsl, in0=ph, in1=bi_bc,
                                        op=mybir.AluOpType.add)
                nc.vector.tensor_scalar(out=hsl, in0=hsl, scalar1=0.0,
                                        scalar2=mask_sb[:, b:b + 1],
                                        op0=mybir.AluOpType.max,
                                        op1=mybir.AluOpType.mult)
                ht = transpose_to_f16(in_ps, hsl)
                nc.vector.tensor_copy(h0T_slice(b), ht)
                ps = in_ps.tile([BLK, hid], F32, tag="u")
                nc.tensor.matmul(ps, lhsT=ht, rhs=w1_sb[:, 0:hid],
                                 start=True, stop=True)
                nc.vector.tensor_copy(supsend[:, b * hid:(b + 1) * hid], ps)
                pi = in_ps.tile([BLK, hid], F32, tag="u")
                nc.tensor.matmul(pi, lhsT=ht, rhs=w2_sb[:, 0:hid],
                                 start=True, stop=True)
                nc.vector.tensor_copy(zinit[:, b * hid:(b + 1) * hid], pi)

        if dbg:
            nc.sync.dma_start(out=dbg_h[:, :], in_=hprev)
            nc.sync.dma_start(out=dbg_sup[:, :], in_=supsend)
        nc.sync.dma_start(out=sup_local.opt(), in_=supsend)
        if NO_COLLECTIVES:
            nc.sync.dma_start(out=tables[0][:shard, :], in_=sup_local.opt())
        else:
            nc.gpsimd.collective_compute(
                "AllGather", mybir.AluOpType.bypass, replica_groups=[core_ids],
                ins=[sup_local.opt()], outs=[tables[0].opt()])

        # ---- layers
        table_wide = None  # built lazily from table AP

        for l in range(nlay):
            lay_ctx = ExitStack()
            agg_ps = lay_ctx.enter_context(
                tc.tile_pool(name=f"agg{l}", bufs=CHUNK_BLKS + 1, space="PSUM"))
            st_ps = lay_ctx.enter_context(
                tc.tile_pool(name=f"st{l}", bufs=1, space="PSUM"))
            st_sum = st_ps.tile([1, hid], F32, tag="ssum")
            st_sq = st_ps.tile([1, hid], F32, tag="ssq")
            evict_cnt = 0
            for ch in cfg["chunks"]:
                g = gpool.tile([BLK, cfg["max_chunk_cols"] * BLK], F16, tag="g")
                tw = tables[l][:, :].rearrange("(w q) f -> w (q f)", q=4)
                for cc2, ca in enumerate(ch["calls"]):
                    a0 = ca["col128_0"] * BLK
                    nc.gpsimd.dma_gather(
                        out_ap=g[:, a0:a0 + ca["num_idxs"]].rearrange(
                            "p (t f) -> p t f", f=BLK),
                        in_ap=tw[:, cc2 * BLK:(cc2 + 1) * BLK],
                        idxs_ap=gidx_sb[:, ca["gi0"]:ca["gi0"] + ca["ncols16"]],
                        num_idxs=ca["num_idxs"],
                        num_idxs_reg=ca["num_idxs"],
                        elem_size=BLK,
                        elem_step=4 * hid,
                        single_packet=(ca["num_idxs"] <= 1024),
                    )
                agg_tiles = {}
                if dbg and l == 0 and ch is cfg["chunks"][0]:
                    nc.sync.dma_start(
                        out=dbg_g[:, :ch["ncol128"] * BLK],
                        in_=g[:, :ch["ncol128"] * BLK])
                for td in ch["tiles"]:
                    gc = td["gcol"]
                    ti = td["tid"]
                    for (b, var, st, sp) in td["flags"]:
                        pt = ppool.tile([BLK, BLK], F16, tag="p")
                        nc.vector.tensor_scalar(
                            out=pt, in0=iotas[var],
                            scalar1=edat_sb[:, ti:ti + 1],
                            scalar2=edat_sb[:, ntiles + ti:ntiles + ti + 1],
                            op0=mybir.AluOpType.is_equal,
                            op1=mybir.AluOpType.mult)
                        if st:
                            agg_tiles[b] = agg_ps.tile([BLK, hid], F32,
                                                       tag="agg",
                                                       name=f"agg_b{b}")
                        nc.tensor.matmul(agg_tiles[b], lhsT=pt,
                                         rhs=g[:, gc:gc + hid],
                                         start=st, stop=sp)
                        if sp:
                            zsl = zinit[:, b * hid:(b + 1) * hid]
                            nc.vector.tensor_tensor(out=zsl, in0=agg_tiles[b],
                                                    in1=zsl,
                                                    op=mybir.AluOpType.add)
                            zsq = tpool.tile([BLK, hid], F32, tag="zsq")
                            nc.vector.tensor_tensor(out=zsq, in0=zsl, in1=zsl,
                                                    op=mybir.AluOpType.mult)
                            nc.tensor.matmul(st_sum, lhsT=ones[:, :1], rhs=zsl,
                                             start=(evict_cnt == 0),
                                             stop=(evict_cnt == nblk - 1))
                            nc.tensor.matmul(st_sq, lhsT=ones[:, :1], rhs=zsq,
                                             start=(evict_cnt == 0),
                                             stop=(evict_cnt == nblk - 1))
                            evict_cnt += 1
                            del agg_tiles[b]

            # stats allreduce
            nc.vector.tensor_copy(stats_sb[:1, 0:hid], st_sum)
            nc.vector.tensor_copy(stats_sb[:1, hid:2 * hid], st_sq)
            lay_ctx.close()
            nc.sync.dma_start(out=stats_in.opt(), in_=stats_sb)
            if NO_COLLECTIVES:
                nc.sync.dma_start(out=stats_sb, in_=stats_in.opt())
            else:
                nc.gpsimd.collective_compute(
                    "AllReduce", mybir.AluOpType.add, replica_groups=[core_ids],
                    ins=[stats_in.opt()], outs=[stats_outs[l].opt()])
                nc.sync.dma_start(out=stats_sb, in_=stats_outs[l].opt())

            mt = tpool.tile([1, hid], F32, tag="mt")
            nc.vector.tensor_scalar(out=mt, in0=stats_sb[:1, 0:hid],
                                    scalar1=inv_n, scalar2=None,
                                    op0=mybir.AluOpType.mult)
            vt = tpool.tile([1, hid], F32, tag="vt")
            nc.vector.tensor_scalar(out=vt, in0=stats_sb[:1, hid:2 * hid],
                                    scalar1=inv_n, scalar2=None,
                                    op0=mybir.AluOpType.mult)
            mm = tpool.tile([1, hid], F32, tag="mm")
            nc.vector.tensor_tensor(out=mm, in0=mt, in1=mt,
                                    op=mybir.AluOpType.mult)
            nc.vector.tensor_tensor(out=vt, in0=vt, in1=mm,
                                    op=mybir.AluOpType.subtract)
            rstd = tpool.tile([1, hid], F32, tag="rstd")
            nc.vector.tensor_scalar(out=vt, in0=vt, scalar1=EPS, scalar2=None,
                                    op0=mybir.AluOpType.add)
            nc.vector.reciprocal(rstd, vt)
            nc.scalar.activation(out=rstd, in_=rstd,
                                 func=mybir.ActivationFunctionType.Sqrt)
            scl = tpool.tile([1, hid], F32, tag="scl")
            nc.vector.tensor_tensor(out=scl, in0=gb_sb[:1, (l * 2) * hid:(l * 2 + 1) * hid],
                                    in1=rstd, op=mybir.AluOpType.mult)
            sht = tpool.tile([1, hid], F32, tag="sht")
            nc.vector.tensor_tensor(out=sht, in0=mt, in1=scl,
                                    op=mybir.AluOpType.mult)
            nc.vector.tensor_tensor(out=sht, in0=gb_sb[:1, (l * 2 + 1) * hid:(l * 2 + 2) * hid],
                                    in1=sht, op=mybir.AluOpType.subtract)
            if dbg and l == 0:
                nc.sync.dma_start(out=dbg_z[:, :], in_=zinit)
                nc.sync.dma_start(out=dbg_st[:, :], in_=stats_sb)
            scl_bc = tpool.tile([BLK, hid], F32, tag="sclbc")
            nc.gpsimd.partition_broadcast(scl_bc, scl[:1, :])
            sht_bc = tpool.tile([BLK, hid], F32, tag="shtbc")
            nc.gpsimd.partition_broadcast(sht_bc, sht[:1, :])

            # update h, produce next support/initial (or final output)
            up_ctx = ExitStack()
            up_ps = up_ctx.enter_context(
                tc.tile_pool(name=f"up{l}", bufs=8, space="PSUM"))
            for b in range(nblk):
                zsl = zinit[:, b * hid:(b + 1) * hid]
                hsl = hprev[:, b * hid:(b + 1) * hid]
                t1_ = tpool.tile([BLK, hid], F32, tag="t1")
                nc.vector.tensor_tensor(out=t1_, in0=zsl, in1=scl_bc,
                                        op=mybir.AluOpType.mult)
                nc.vector.tensor_tensor(out=t1_, in0=t1_, in1=sht_bc,
                                        op=mybir.AluOpType.add)
                nc.vector.tensor_scalar(out=t1_, in0=t1_, scalar1=0.0,
                                        scalar2=None, op0=mybir.AluOpType.max)
                nc.vector.tensor_tensor(out=hsl, in0=t1_, in1=hsl,
                                        op=mybir.AluOpType.add)
                ht = transpose_to_f16(up_ps, hsl)
                if l < nlay - 1:
                    ps = up_ps.tile([BLK, hid], F32, tag="u")
                    nc.tensor.matmul(ps, lhsT=ht,
                                     rhs=w1_sb[:, (l + 1) * hid:(l + 2) * hid],
                                     start=True, stop=True)
                    nc.vector.tensor_copy(supsend[:, b * hid:(b + 1) * hid], ps)
                    pi = up_ps.tile([BLK, hid], F32, tag="u")
                    nc.tensor.matmul(pi, lhsT=h0T_slice(b),
                                     rhs=w2_sb[:, (l + 1) * hid:(l + 2) * hid],
                                     start=True, stop=True)
                    nc.vector.tensor_copy(zinit[:, b * hid:(b + 1) * hid], pi)
                else:
                    po = up_ps.tile([BLK, outp], F32, tag="u")
                    nc.tensor.matmul(po, lhsT=ht[:, :], rhs=wo_sb,
                                     start=True, stop=True)
                    nc.vector.tensor_tensor(
                        out=outsb[:, b * outp:(b + 1) * outp], in0=po,
                        in1=bo_bc[:, :outp], op=mybir.AluOpType.add)
            up_ctx.close()
            if l < nlay - 1:
                nc.sync.dma_start(out=sup_local.opt(), in_=supsend)
                if NO_COLLECTIVES:
                    nc.sync.dma_start(out=tables[l + 1][:shard, :],
                                      in_=sup_local.opt())
                else:
                    nc.gpsimd.collective_compute(
                        "AllGather", mybir.AluOpType.bypass,
                        replica_groups=[core_ids],
                        ins=[sup_local.opt()], outs=[tables[l + 1].opt()])

        nc.sync.dma_start(out=out_p[:, :], in_=outsb)
    return nc


# ------------------------------------------------------------- timed runner
def _run_spmd_timed(nc, in_maps, n_cores, repeats):
    """Mirror of bass2jax.run_bass_via_pjrt with repeat timing (no donation,
    inputs pre-staged on device)."""
    import jax
    import time
    from jax.sharding import Mesh, PartitionSpec, NamedSharding
    from jax.experimental.shard_map import shard_map
    from concourse import bass2jax

    bass2jax.install_neuronx_cc_hook()
    partition_name = (nc.partition_id_tensor.name
                     if nc.partition_id_tensor else None)
    in_names, out_names, out_avals, zero_outs = [], [], [], []
    for alloc in nc.m.functions[0].allocations:
        if not isinstance(alloc, mybir.MemoryLocationSet):
            continue
        name = alloc.memorylocations[0].name
        if alloc.kind == "ExternalInput":
            if name != partition_name:
                in_names.append(name)
        elif alloc.kind == "ExternalOutput":
            shape = tuple(alloc.tensor_shape)
            dtype = mybir.dt.np(alloc.dtype)
            out_names.append(name)
            out_avals.append(jax.core.ShapedArray(shape, dtype))
            zero_outs.append(np.zeros(shape, dtype))
    n_params = len(in_names)
    in_names_full = list(in_names) + list(out_names)
    if partition_name is not None:
        in_names_full.append(partition_name)

    def _body(*args):
        operands = list(args)
        if partition_name is not None:
            operands.append(bass2jax.partition_id_tensor())
        outs = bass2jax._bass_exec_p.bind(
            *operands, out_avals=tuple(out_avals),
            in_names=tuple(in_names_full), out_names=tuple(out_names),
            lowering_input_output_aliases=(),
            sim_require_finite=True, sim_require_nnan=True, nc=nc)
        return tuple(outs)

    devices = jax.devices()[:n_cores]
    mesh = Mesh(np.asarray(devices), ("core",))
    spec = PartitionSpec("core")
    n_outs = len(out_avals)
    sharded = jax.jit(shard_map(
        _body, mesh=mesh, in_specs=(spec,) * (n_params + n_outs),
        out_specs=(spec,) * n_outs, check_rep=False), keep_unused=True)
    concat_in = [
        np.concatenate([np.asarray(in_maps[c][nm]) for c in range(n_cores)],
                       axis=0)
        for nm in in_names]
    concat_zeros = [np.zeros((n_cores * z.shape[0], *z.shape[1:]), z.dtype)
                    for z in zero_outs]
    sh = NamedSharding(mesh, spec)
    dev_in = [jax.device_put(a, sh) for a in concat_in + concat_zeros]
    for a in dev_in:
        a.block_until_ready()
    out_arrs = sharded(*dev_in)
    jax.block_until_ready(out_arrs)
    times = []
    for _ in range(repeats):
        t0 = time.perf_counter()
        o = sharded(*dev_in)
        jax.block_until_ready(o)
        times.append(time.perf_counter() - t0)
    exec_ns = int(min(times) * 1e9) if times else None
    results = [
        {nm: np.asarray(out_arrs[i]).reshape(
            n_cores, *out_avals[i].shape)[c]
         for i, nm in enumerate(out_names)}
        for c in range(n_cores)]
    return results, exec_ns, times


# ---------------------------------------------------------------- entry
def kernel(**inputs):
    x = np.asarray(inputs["x"], np.float32)
    edge_index = np.asarray(inputs["edge_index"])
    Wi = np.asarray(inputs["Wi"], np.float32)
    bi = np.asarray(inputs["bi"], np.float32)
    w1 = np.asarray(inputs["w1"], np.float32)
    w2 = np.asarray(inputs["w2"], np.float32)
    gamma = np.asarray(inputs["gamma"], np.float32)
    beta = np.asarray(inputs["beta"], np.float32)
    Wo = np.asarray(inputs["Wo"], np.float32)
    bo = np.asarray(inputs["bo"], np.float32)

    n_nodes, fin = x.shape
    hid = Wi.shape[1]
    nlay = w1.shape[0]
    outd = Wo.shape[1]
    nblk = -(-n_nodes // (NC * BLK))
    shard = nblk * BLK
    npad = NC * shard

    assert npad % 4 == 0
    prep = _host_prep(edge_index, n_nodes, npad, shard, nblk, CHUNK_BLKS)
    ntiles = prep["ntiles"]
    max_chunk_cols = max(ch["ncol128"] for ch in prep["chunks"])

    cfg = dict(fin=fin, hid=hid, outd=outd, nlay=nlay, shard=shard, nblk=nblk,
               npad=npad, n=n_nodes, ntiles=ntiles, chunks=prep["chunks"],
               gcols=prep["gcols"], max_chunk_cols=max_chunk_cols, debug=DEBUG)

    # host-side tensor prep
    xpad = np.zeros((npad, fin), np.float32)
    xpad[:n_nodes] = x
    w1p = (w1 + np.eye(hid, dtype=np.float32)).astype(np.float16)
    w2p = (w2 + ALPHA * np.eye(hid, dtype=np.float32)).astype(np.float16)
    gb = np.stack([gamma, beta], axis=1).astype(np.float32)      # [L,2,H]
    bvec = np.zeros((2, hid), np.float32)
    bvec[0] = bi
    bvec[1, :outd] = bo
    mask = np.zeros((npad,), np.float32)
    mask[:n_nodes] = 1.0

    in_maps = []
    for c in range(NC):
        xs = xpad[c * shard:(c + 1) * shard].astype(np.float16)
        mk = mask[c * shard:(c + 1) * shard].reshape(nblk, BLK).T.copy()
        in_maps.append({
            "xT": np.ascontiguousarray(xs.T),
            "gidx": prep["gidx"][c],
            "edat": prep["edat"][c],
            "wi": Wi.astype(np.float16),
            "w1p": w1p, "w2p": w2p, "gb": gb,
            "wo": Wo.astype(np.float16), "bvec": bvec,
            "msk": np.ascontiguousarray(mk),
        })

    nc = bacc.Bacc("TRN2", target_bir_lowering=False, debug=False,
                   num_devices=NC)
    _build(nc, cfg)
    nc.compile()
    if BUILD_ONLY:
        return nc, in_maps
    global LAST, EXEC_NS, EXEC_TIMES
    if TIME_REPEATS > 0:
        results, EXEC_NS, EXEC_TIMES = _run_spmd_timed(
            nc, in_maps, NC, TIME_REPEATS)

        class _R:
            pass
        res = _R()
        res.results = results
        res.exec_time_ns = EXEC_NS
        res.mean_exec_time_ns = None
        LAST = res
    else:
        res = run_bass_kernel_spmd(nc, in_maps, list(range(NC)),
                                   trace=TRACE, tmpdir=TRACE_DIR)
        LAST = res

    outp = max(outd, 1)
    parts = []
    for c in range(NC):
        arr = res.results[c]["out"]                    # [128, nblk*outp]
        arr = arr.reshape(BLK, nblk, outp).transpose(1, 0, 2).reshape(shard, outp)
        parts.append(arr[:, :outd])
    full = np.concatenate(parts, axis=0)[:n_nodes]
    return full.astype(np.float32)
